# revision 9
# baseline (speedup 1.0000x reference)
"""Trainium2 Bass kernel for the crossbar-MVM quantized Conv2d.

The reference's analog-crossbar emulation (bit-sliced weights, bit-streamed
inputs, conductance mapping, per-column ADC) is exactly equivalent to a
fixed-point quantized conv:

    Wq  = rne(w * 64)                       (pos/neg split recombined; the
                                             +-255 clip never binds: |w*64|<=~15)
    Xq  = clip(rne(x * 64), -128, 127)
    out = clip((im2col(Xq) @ Wq.T) * 2^-12, -8.0, 8.0 - 2^-12)

because the ADC never saturates (max column sum 3*128=384 < 2^9-1) and the
conductance mapping is exactly invertible: the f32 einsum error (~1e-4) is far
below the 0.5 rounding margin, so round() recovers the exact integer dot
product for any accumulation order.  All arithmetic here is exact: rne via the
1.5*2^23 magic constant in f32, Wq*2^-12 and Xq exact in bf16, products and
sums exact in f32 PSUM (< 2^24), so the result is bit-identical to the
reference.

Sharding: data-parallel over batch (8 batches -> 8 cores), weight replicated.

This kernel is latency-bound (a trivial DMA-in/DMA-out NEFF measures ~13.6us:
program preamble + per-DMA trigger/descriptor/semaphore latency + a fixed
~7.7us sequencer teardown), so the structure minimizes the serial chain
between "weights landed" and "output DMA issued":

- The two input DMAs are triggered concurrently on the two HWDGE queue
  engines (x on Sync, w on Activation) instead of serially on Sync.
- Weight quantization runs split in column halves across Activation and DVE
  (activation Copy computes in*scale+bias, exact for these dyadic constants),
  halving each step's latency.
- The input is stored ROW-padded ([18,16] with zero top/bottom rows) in the
  lower partition half of one [128, 288] tile, and the same data shifted by
  one row in the upper half; both are plain contiguous stores of the final
  quant step (memset supplies the zeros).  Kernel taps that differ only in
  the row offset i then share their column-validity window, so tap pairs
  (0,3),(1,4),(2,5) each run as ONE K=128 matmul against the stacked tile,
  and taps 6/7/8 run at K=64 (tap 7 against the upper half).  Column edges
  are handled by accumulating into column sub-rectangles of PSUM (the full
  j=1 pair opens the accumulation group).  Net: 5 pair-transposes + 6
  matmuls on PE instead of 9 + 9.
- The final clamp is split across DVE/GpSimd by output-channel half and the
  output DMA across Sync/Activation, overlapping descriptor generation.
"""

import numpy as np

import concourse.bacc as bacc
import concourse.bass as bass
import concourse.mybir as mybir
import concourse.tile as tile
from concourse.bass_utils import run_bass_kernel_spmd
from concourse.masks import make_identity

N_CORES = 8
B, CIN, H, W = 8, 64, 16, 16
COUT, KH, KW = 128, 3, 3
PIX = H * W
MAGIC = 12582912.0  # 1.5 * 2^23: f32 add/sub rounds to nearest-even integer
OUT_SCALE = 2.0**-12
ACM_LO = -8.0
ACM_HI = 8.0 - 2.0**-12

_ALU = mybir.AluOpType
_ACT = mybir.ActivationFunctionType
_F32 = mybir.dt.float32
_BF16 = mybir.dt.bfloat16

L = CIN * KH * KW  # 576
HALF = L // 2  # 288
PADPIX = (H + 2) * W  # 288: row-padded image, flat [18, 16]


def _build_nc() -> bass.Bass:
    # Bacc (not raw Bass): its compile() pass splits multi-sem waits into
    # event-semaphore chains — walrus rejects >1 sync wait per instruction.
    nc = bacc.Bacc(trn_type="TRN2")
    x_d = nc.declare_dram_parameter("x", [1, CIN, H, W], _F32, isOutput=False)
    w_d = nc.declare_dram_parameter("weight", [COUT, CIN, KH, KW], _F32, isOutput=False)
    o_d = nc.declare_dram_parameter("out", [1, COUT, H, W], _F32, isOutput=True)

    with tile.TileContext(nc) as tc:
        with (
            tc.tile_pool(name="sbuf", bufs=1) as pool,
            tc.tile_pool(name="tpsum", bufs=3, space="PSUM") as tpsum,
            tc.tile_pool(name="apsum", bufs=1, space="PSUM") as apsum,
        ):
            # ---- input DMAs first, on separate HWDGE engines, so both
            # trigger concurrently at program start.
            xs = pool.tile([CIN, PIX], _F32)
            nc.sync.dma_start(xs[:], x_d.rearrange("b c h w -> (b c) (h w)"))
            ws = pool.tile([COUT, L], _F32)
            nc.scalar.dma_start(ws[:], w_d.rearrange("co ci kh kw -> co (ci kh kw)"))

            # ---- dependency-free setup (GpSimd): stacked-input tile zeros
            # (supplies the row padding) and the transpose identity.
            x2r = pool.tile([128, PADPIX], _BF16)
            nc.gpsimd.memset(x2r[:], 0.0)
            ident = pool.tile([128, 128], _BF16)
            make_identity(nc, ident[:])

            # ---- input quant: Xq = clip(rne(x*64), -128, 127) in bf16.
            # Final subtract-MAGIC step lands directly in the stacked tile:
            # lower half rows 1..16 of the padded [18,16] image, upper half
            # the same data one row up (so upper[t] == lower[t+16]).
            x1 = pool.tile([CIN, PIX], _F32)
            nc.vector.tensor_scalar(x1[:], xs[:], 64.0, MAGIC, _ALU.mult, _ALU.add)
            x2 = pool.tile([CIN, PIX], _F32)
            nc.vector.tensor_scalar(
                x2[:], x1[:], MAGIC - 128.0, MAGIC + 127.0, _ALU.max, _ALU.min
            )
            nc.vector.tensor_scalar(
                x2r[0:CIN, W : W + PIX], x2[:], MAGIC, None, _ALU.subtract
            )
            nc.gpsimd.tensor_scalar(
                x2r[CIN:128, 0:PIX], x2[:], MAGIC, None, _ALU.subtract
            )

            # ---- weight quant: Wq*2^-12 in bf16 (exact), each step split in
            # column halves across Activation and DVE.
            wt = pool.tile([COUT, L], _F32)
            nc.gpsimd.tensor_scalar(
                wt[:, 0:HALF], ws[:, 0:HALF], 64.0, MAGIC, _ALU.mult, _ALU.add
            )
            nc.vector.tensor_scalar(
                wt[:, HALF:L], ws[:, HALF:L], 64.0, MAGIC, _ALU.mult, _ALU.add
            )
            wq = pool.tile([COUT, L], _BF16)
            nc.vector.tensor_scalar(
                wq[:, 0:HALF], wt[:, 0:HALF], MAGIC, OUT_SCALE, _ALU.subtract, _ALU.mult
            )
            nc.gpsimd.tensor_scalar(
                wq[:, HALF:L], wt[:, HALF:L], MAGIC, OUT_SCALE, _ALU.subtract, _ALU.mult
            )

            # ---- per-tap PE transposes.  wq free layout is (ci, i, j); the
            # walrus verifier allows only ONE free dim on the stationary AP,
            # so taps transpose individually — but each pair (i=0,j)/(i=1,j)
            # lands in the top/bottom partition halves of one PSUM tile
            # (out base partition 64 is legal for col_size 64), so the pair
            # still needs only one SBUF copy and one K=128 matmul.
            wq4 = wq[:].rearrange("co (ci i j) -> co ci i j", i=KH, j=KW)

            w14 = pool.tile([128, COUT], _BF16)  # taps 1,4 (j=1)
            w03 = pool.tile([128, COUT], _BF16)  # taps 0,3 (j=0)
            w25 = pool.tile([128, COUT], _BF16)  # taps 2,5 (j=2)
            w6 = pool.tile([CIN, COUT], _BF16)  # tap 6 (i=2, j=0)
            w7 = pool.tile([CIN, COUT], _BF16)  # tap 7 (i=2, j=1)
            w8 = pool.tile([CIN, COUT], _BF16)  # tap 8 (i=2, j=2)

            def tap_T(i, j, dst):
                pt = tpsum.tile([CIN, COUT], _BF16, tag="pt")
                nc.tensor.transpose(pt[:], wq4[:, :, i, j], ident[:])
                nc.vector.tensor_copy(dst, pt[:])

            tap_T(0, 1, w14[0:CIN, :])
            tap_T(1, 1, w14[CIN:128, :])
            tap_T(0, 0, w03[0:CIN, :])
            tap_T(1, 0, w03[CIN:128, :])
            tap_T(0, 2, w25[0:CIN, :])
            tap_T(1, 2, w25[CIN:128, :])
            tap_T(2, 0, w6[:])
            tap_T(2, 1, w7[:])
            tap_T(2, 2, w8[:])

            # ---- conv: 6 accumulating matmuls.  x2r viewed [p, 18, 16]:
            # window rows [i:i+16] hits taps (i, j) on the lower half and
            # (i+1, j) on the upper.  Column edges (j=0 writes out cols 1:,
            # j=2 cols :-1) accumulate into PSUM sub-rectangles; the full
            # j=1 pair opens the group.
            xv = x2r[:].rearrange("p (h w) -> p h w", w=W)
            acc = apsum.tile([COUT, H, W], _F32)
            nc.tensor.matmul(acc[:], w14[:], xv[:, 0:16, :], start=True, stop=False)
            nc.tensor.matmul(
                acc[:, :, 1:W], w03[:], xv[:, 0:16, 0 : W - 1], start=False, stop=False
            )
            nc.tensor.matmul(
                acc[:, :, 0 : W - 1], w25[:], xv[:, 0:16, 1:W], start=False, stop=False
            )
            nc.tensor.matmul(
                acc[:, :, 1:W],
                w6[:],
                xv[0:CIN, 2:18, 0 : W - 1],
                start=False,
                stop=False,
            )
            nc.tensor.matmul(
                acc[:],
                w7[:],
                xv[0:CIN, 2:18, :],
                start=False,
                stop=False,
            )
            nc.tensor.matmul(
                acc[:, :, 0 : W - 1],
                w8[:],
                xv[0:CIN, 2:18, 1:W],
                start=False,
                stop=True,
            )

            # ---- epilogue: clamp in co halves (DVE only — GpSimd cannot
            # access PSUM), store with two DMAs on separate HWDGE engines so
            # the first half's descriptors generate while the second clamps.
            av = acc[:].rearrange("co h w -> co (h w)")
            ob = pool.tile([COUT, PIX], _F32)
            nc.vector.tensor_scalar(
                ob[:], av[:], ACM_LO, ACM_HI, _ALU.max, _ALU.min
            )
            ov = o_d.rearrange("b c h w -> (b c) (h w)")
            nc.sync.dma_start(ov[0:64, :], ob[0:64, :])
            nc.scalar.dma_start(ov[64:128, :], ob[64:128, :])

    # Bacc defers register allocation to finalize()/compile(); the PJRT spmd
    # path serializes nc.m without finalizing, so do it here.
    nc.finalize()
    return nc


_NC_CACHE: bass.Bass | None = None


def _get_nc() -> bass.Bass:
    global _NC_CACHE
    if _NC_CACHE is None:
        _NC_CACHE = _build_nc()
    return _NC_CACHE


def _run(x: np.ndarray, weight: np.ndarray, **spmd_kwargs):
    x = np.ascontiguousarray(np.asarray(x, dtype=np.float32))
    weight = np.ascontiguousarray(np.asarray(weight, dtype=np.float32))
    assert x.shape == (B, CIN, H, W), x.shape
    assert weight.shape == (COUT, CIN, KH, KW), weight.shape

    in_maps = [{"x": x[b : b + 1], "weight": weight} for b in range(N_CORES)]
    res = run_bass_kernel_spmd(_get_nc(), in_maps, list(range(N_CORES)), **spmd_kwargs)
    out = np.concatenate([res.results[c]["out"] for c in range(N_CORES)], axis=0)
    return out, res


def kernel(x: np.ndarray, weight: np.ndarray) -> np.ndarray:
    out, _ = _run(x, weight)
    return out


# revision 10
# speedup vs baseline: 1.4069x; 1.4069x over previous
"""Trainium2 Bass kernel for the crossbar-MVM quantized Conv2d.

The reference's analog-crossbar emulation (bit-sliced weights, bit-streamed
inputs, conductance mapping, per-column ADC) is exactly equivalent to a
fixed-point quantized conv:

    Wq  = rne(w * 64)                       (pos/neg split recombined; the
                                             +-255 clip never binds: |w*64|<=~15)
    Xq  = clip(rne(x * 64), -128, 127)
    out = clip((im2col(Xq) @ Wq.T) * 2^-12, -8.0, 8.0 - 2^-12)

because the ADC never saturates (max column sum 3*128=384 < 2^9-1) and the
conductance mapping is exactly invertible: the f32 einsum error (~1e-4) is far
below the 0.5 rounding margin, so round() recovers the exact integer dot
product for any accumulation order.  All arithmetic here is exact: rne via the
1.5*2^23 magic constant in f32, Wq*2^-12 and Xq exact in bf16, products and
sums exact in f32 PSUM (< 2^24), so the result is bit-identical to the
reference.

Sharding: data-parallel over batch (8 batches -> 8 cores), weight replicated.

This kernel is latency-bound (a trivial DMA-in/DMA-out NEFF measures ~13.6us:
program preamble + per-DMA trigger/descriptor/semaphore latency + a fixed
~7.7us sequencer teardown), so the structure minimizes the serial chain
between "weights landed" and "output DMA issued":

- The two input DMAs are triggered concurrently on the two HWDGE queue
  engines (x on Sync, w on Activation) instead of serially on Sync.
- Weight quantization runs split in column halves across Activation and DVE
  (activation Copy computes in*scale+bias, exact for these dyadic constants),
  halving each step's latency.
- The input is stored ROW-padded ([18,16] with zero top/bottom rows) in the
  lower partition half of one [128, 288] tile, and the same data shifted by
  one row in the upper half; both are plain contiguous stores of the final
  quant step (memset supplies the zeros).  Kernel taps that differ only in
  the row offset i then share their column-validity window, so tap pairs
  (0,3),(1,4),(2,5) each run as ONE K=128 matmul against the stacked tile,
  and taps 6/7/8 run at K=64 (tap 7 against the upper half).  Column edges
  are handled by accumulating into column sub-rectangles of PSUM (the full
  j=1 pair opens the accumulation group).  Net: 5 pair-transposes + 6
  matmuls on PE instead of 9 + 9.
- The final clamp is split across DVE/GpSimd by output-channel half and the
  output DMA across Sync/Activation, overlapping descriptor generation.
"""

import numpy as np

import concourse.bacc as bacc
import concourse.bass as bass
import concourse.mybir as mybir
import concourse.tile as tile
from concourse.bass_utils import run_bass_kernel_spmd
from concourse.masks import make_identity

N_CORES = 8
B, CIN, H, W = 8, 64, 16, 16
COUT, KH, KW = 128, 3, 3
PIX = H * W
MAGIC = 12582912.0  # 1.5 * 2^23: f32 add/sub rounds to nearest-even integer
OUT_SCALE = 2.0**-12
ACM_LO = -8.0
ACM_HI = 8.0 - 2.0**-12

_ALU = mybir.AluOpType
_ACT = mybir.ActivationFunctionType
_F32 = mybir.dt.float32
_BF16 = mybir.dt.bfloat16

L = CIN * KH * KW  # 576
HALF = L // 2  # 288
PADPIX = (H + 2) * W  # 288: row-padded image, flat [18, 16]


def _build_nc() -> bass.Bass:
    # Bacc (not raw Bass): its compile() pass splits multi-sem waits into
    # event-semaphore chains — walrus rejects >1 sync wait per instruction.
    nc = bacc.Bacc(trn_type="TRN2")
    x_d = nc.declare_dram_parameter("x", [1, CIN, H, W], _F32, isOutput=False)
    w_d = nc.declare_dram_parameter("weight", [COUT, CIN, KH, KW], _F32, isOutput=False)
    o_d = nc.declare_dram_parameter("out", [1, COUT, H, W], _F32, isOutput=True)

    with tile.TileContext(nc) as tc:
        with (
            tc.tile_pool(name="sbuf", bufs=1) as pool,
            tc.tile_pool(name="tpsum", bufs=3, space="PSUM") as tpsum,
            tc.tile_pool(name="apsum", bufs=1, space="PSUM") as apsum,
        ):
            # ---- input DMAs first, on separate HWDGE engines, so both
            # trigger concurrently at program start.
            xs = pool.tile([CIN, PIX], _F32)
            nc.sync.dma_start(xs[:], x_d.rearrange("b c h w -> (b c) (h w)"))
            ws = pool.tile([COUT, L], _F32)
            nc.scalar.dma_start(ws[:], w_d.rearrange("co ci kh kw -> co (ci kh kw)"))

            # ---- dependency-free setup (GpSimd): stacked-input tile zeros
            # (supplies the row padding) and the transpose identity.
            x2r = pool.tile([128, PADPIX], _BF16)
            nc.gpsimd.memset(x2r[:], 0.0)
            ident = pool.tile([128, 128], _BF16)
            make_identity(nc, ident[:])

            # ---- input quant: Xq = clip(rne(x*64), -128, 127) in bf16.
            # Final subtract-MAGIC step lands directly in the stacked tile:
            # lower half rows 1..16 of the padded [18,16] image, upper half
            # the same data one row up (so upper[t] == lower[t+16]).
            x1 = pool.tile([CIN, PIX], _F32)
            nc.vector.tensor_scalar(x1[:], xs[:], 64.0, MAGIC, _ALU.mult, _ALU.add)
            x2 = pool.tile([CIN, PIX], _F32)
            nc.vector.tensor_scalar(
                x2[:], x1[:], MAGIC - 128.0, MAGIC + 127.0, _ALU.max, _ALU.min
            )
            nc.vector.tensor_scalar(
                x2r[0:CIN, W : W + PIX], x2[:], MAGIC, None, _ALU.subtract
            )
            nc.vector.tensor_scalar(
                x2r[CIN:128, 0:PIX], x2[:], MAGIC, None, _ALU.subtract
            )

            # ---- weight quant: Wq*2^-12 in bf16 (exact), each step split in
            # column halves across Activation and DVE.
            wt = pool.tile([COUT, L], _F32)
            nc.vector.tensor_scalar(wt[:], ws[:], 64.0, MAGIC, _ALU.mult, _ALU.add)
            wq = pool.tile([COUT, L], _BF16)
            nc.vector.tensor_scalar(
                wq[:], wt[:], MAGIC, OUT_SCALE, _ALU.subtract, _ALU.mult
            )

            # ---- per-tap PE transposes.  wq free layout is (ci, i, j); the
            # walrus verifier allows only ONE free dim on the stationary AP,
            # so taps transpose individually — but each pair (i=0,j)/(i=1,j)
            # lands in the top/bottom partition halves of one PSUM tile
            # (out base partition 64 is legal for col_size 64), so the pair
            # still needs only one SBUF copy and one K=128 matmul.
            wq4 = wq[:].rearrange("co (ci i j) -> co ci i j", i=KH, j=KW)

            w14 = pool.tile([128, COUT], _BF16)  # taps 1,4 (j=1)
            w03 = pool.tile([128, COUT], _BF16)  # taps 0,3 (j=0)
            w25 = pool.tile([128, COUT], _BF16)  # taps 2,5 (j=2)
            w6 = pool.tile([CIN, COUT], _BF16)  # tap 6 (i=2, j=0)
            w7 = pool.tile([CIN, COUT], _BF16)  # tap 7 (i=2, j=1)
            w8 = pool.tile([CIN, COUT], _BF16)  # tap 8 (i=2, j=2)

            def tap_T(i, j, dst):
                pt = tpsum.tile([CIN, COUT], _BF16, tag="pt")
                nc.tensor.transpose(pt[:], wq4[:, :, i, j], ident[:])
                nc.vector.tensor_copy(dst, pt[:])

            tap_T(0, 1, w14[0:CIN, :])
            tap_T(1, 1, w14[CIN:128, :])
            tap_T(0, 0, w03[0:CIN, :])
            tap_T(1, 0, w03[CIN:128, :])
            tap_T(0, 2, w25[0:CIN, :])
            tap_T(1, 2, w25[CIN:128, :])
            tap_T(2, 0, w6[:])
            tap_T(2, 1, w7[:])
            tap_T(2, 2, w8[:])

            # ---- conv: 6 accumulating matmuls.  x2r viewed [p, 18, 16]:
            # window rows [i:i+16] hits taps (i, j) on the lower half and
            # (i+1, j) on the upper.  Column edges (j=0 writes out cols 1:,
            # j=2 cols :-1) accumulate into PSUM sub-rectangles; the full
            # j=1 pair opens the group.
            xv = x2r[:].rearrange("p (h w) -> p h w", w=W)
            acc = apsum.tile([COUT, H, W], _F32)
            nc.tensor.matmul(acc[:], w14[:], xv[:, 0:16, :], start=True, stop=False)
            nc.tensor.matmul(
                acc[:, :, 1:W], w03[:], xv[:, 0:16, 0 : W - 1], start=False, stop=False
            )
            nc.tensor.matmul(
                acc[:, :, 0 : W - 1], w25[:], xv[:, 0:16, 1:W], start=False, stop=False
            )
            nc.tensor.matmul(
                acc[:, :, 1:W],
                w6[:],
                xv[0:CIN, 2:18, 0 : W - 1],
                start=False,
                stop=False,
            )
            nc.tensor.matmul(
                acc[:],
                w7[:],
                xv[0:CIN, 2:18, :],
                start=False,
                stop=False,
            )
            nc.tensor.matmul(
                acc[:, :, 0 : W - 1],
                w8[:],
                xv[0:CIN, 2:18, 1:W],
                start=False,
                stop=True,
            )

            # ---- epilogue: clamp in co halves (DVE only — GpSimd cannot
            # access PSUM), store with two DMAs on separate HWDGE engines so
            # the first half's descriptors generate while the second clamps.
            av = acc[:].rearrange("co h w -> co (h w)")
            ob = pool.tile([COUT, PIX], _F32)
            nc.vector.tensor_scalar(
                ob[:], av[:], ACM_LO, ACM_HI, _ALU.max, _ALU.min
            )
            ov = o_d.rearrange("b c h w -> (b c) (h w)")
            nc.sync.dma_start(ov[0:64, :], ob[0:64, :])
            nc.scalar.dma_start(ov[64:128, :], ob[64:128, :])

    # Bacc defers register allocation to finalize()/compile(); the PJRT spmd
    # path serializes nc.m without finalizing, so do it here.
    nc.finalize()
    return nc


_NC_CACHE: bass.Bass | None = None


def _get_nc() -> bass.Bass:
    global _NC_CACHE
    if _NC_CACHE is None:
        _NC_CACHE = _build_nc()
    return _NC_CACHE


def _run(x: np.ndarray, weight: np.ndarray, **spmd_kwargs):
    x = np.ascontiguousarray(np.asarray(x, dtype=np.float32))
    weight = np.ascontiguousarray(np.asarray(weight, dtype=np.float32))
    assert x.shape == (B, CIN, H, W), x.shape
    assert weight.shape == (COUT, CIN, KH, KW), weight.shape

    in_maps = [{"x": x[b : b + 1], "weight": weight} for b in range(N_CORES)]
    res = run_bass_kernel_spmd(_get_nc(), in_maps, list(range(N_CORES)), **spmd_kwargs)
    out = np.concatenate([res.results[c]["out"] for c in range(N_CORES)], axis=0)
    return out, res


def kernel(x: np.ndarray, weight: np.ndarray) -> np.ndarray:
    out, _ = _run(x, weight)
    return out
